# revision 17
# baseline (speedup 1.0000x reference)
"""GCN layer kernel for Trainium2, 8-core SPMD.

Computes: out = (A @ (X @ W + b)) / colsum(A)[:, None],  A = (adj != 0)
with N=8192 nodes, F_in=F_out=512, across 8 NeuronCores.

Sharding: row-shard adjacency and node features (1024 rows per core),
replicate W/b. Each core computes its projected-hidden block, all-gathers
the full hidden (bf16) across the chip, then owns its output row block.
Degree (column sums of A) needs rows from every core: each core computes
partial column sums for free via the binarize op's accumulate output, a
second all-gather shares them, and an on-device tree sum + reciprocal
finishes the normalization.

Performance notes (measured on hw):
- Collective activity (entry barrier + AllGather) throttles regular DMA
  traffic, so the A-tile stream is prefetched deeply (a_bin is bf16 to
  make ~36 tiles fit in SBUF) and all pools coexist (a pool teardown
  between phases was serializing the A stream behind phase-1 epilogue).
- The HAM clock gate drops the PE to 1.2 GHz after ~3.4us idle; the
  barrier+gather window is bridged with slow f32 (4-pass) junk matmuls
  whose results are discarded by the first real matmul's start=True.
- All heavy matmuls are bf16: A entries are exactly 0/1 so the lhsT is
  exact; only H carries bf16 rounding (~1.6e-3 final rel err).
"""
import numpy as np

N = 8192
F = 512
N_CORES = 8
NB = N // N_CORES          # 1024 rows per core
KT = N // 128              # 64 contraction tiles
MT = NB // 128             # 8 output row tiles per core
FI_T = F // 128            # 4 feat-in tiles
N_DUMMY = 100               # junk warm-up matmuls (~1.1us each, f32 4-pass)

_cached = {}


def _build():
    import concourse.bacc as bacc
    import concourse.bass as bass
    import concourse.tile as tile
    from concourse import mybir

    f32 = mybir.dt.float32
    bf16 = mybir.dt.bfloat16

    nc = bacc.Bacc("TRN2", target_bir_lowering=False, debug=False,
                   num_devices=N_CORES)
    at = nc.dram_tensor("at", [N, NB], f32, kind="ExternalInput").ap()
    xt = nc.dram_tensor("xt", [F, NB], f32, kind="ExternalInput").ap()
    w = nc.dram_tensor("w", [F, F], f32, kind="ExternalInput").ap()
    bfull = nc.dram_tensor("bfull", [128, F], f32, kind="ExternalInput").ap()
    out = nc.dram_tensor("out", [NB, F], f32, kind="ExternalOutput").ap()

    pid = nc.partition_id()

    with tile.TileContext(nc) as tc:
        with tc.tile_pool(name="dram", bufs=1, space="DRAM") as dram, \
             tc.tile_pool(name="p", bufs=1) as p, \
             tc.tile_pool(name="ps", bufs=1, space="PSUM") as ps:
            hg_in = dram.tile([NB, F], bf16)
            hg_out = dram.tile([N, F], bf16, addr_space="Shared")
            dg_in = dram.tile([128, KT], f32)
            dg_out = dram.tile([128 * N_CORES, KT], f32, addr_space="Shared")

            cs = p.tile([128, KT], f32)    # per-core partial column sums
            ones = p.tile([128, NB], f32)
            nc.vector.memset(ones[:], 1.0)

            # ---- phase 1: H_blk = X_blk @ W + b  (bf16 matmuls) ----
            # stage f32 chunks through small buffers, cast to bf16
            xt_c = p.tile([128, FI_T * NB], bf16)
            w_c = p.tile([128, FI_T * F], bf16)
            for ki in range(FI_T):
                stg_x = p.tile([128, NB], f32, tag="stgx", bufs=2,
                               name=f"stgx{ki}")
                nc.sync.dma_start(stg_x[:], xt[ki * 128:(ki + 1) * 128, :])
                nc.vector.tensor_copy(xt_c[:, ki * NB:(ki + 1) * NB], stg_x[:])
                stg_w = p.tile([128, F], f32, tag="stgw", bufs=2,
                               name=f"stgw{ki}")
                nc.sync.dma_start(stg_w[:], w[ki * 128:(ki + 1) * 128, :])
                nc.vector.tensor_copy(w_c[:, ki * F:(ki + 1) * F], stg_w[:])
            b_sb = p.tile([128, F], f32)
            nc.sync.dma_start(b_sb[:], bfull)

            # single PSUM pool: 8 banks, all held by the main accumulators;
            # phase 1 and warm-up reuse them as scratch (the first real
            # matmul's start=True clears each bank).
            pms = []
            for m in range(MT):
                pm = ps.tile([128, F], f32, tag=f"pm{m}", name=f"pm{m}",
                             bufs=1)
                pms.append(pm)

            for nt in range(MT):
                hp = pms[nt % 2]
                for ki in range(FI_T):
                    nc.tensor.matmul(
                        hp[:],
                        xt_c[:, ki * NB + nt * 128: ki * NB + (nt + 1) * 128],
                        w_c[:, ki * F:(ki + 1) * F],
                        start=(ki == 0), stop=(ki == FI_T - 1))
                hb = p.tile([128, F], bf16, tag="hb", bufs=2, name=f"hb{nt}")
                nc.vector.tensor_tensor(hb[:], hp[:], b_sb[:],
                                        mybir.AluOpType.add)
                nc.gpsimd.dma_start(hg_in[nt * 128:(nt + 1) * 128, :], hb[:])

            # ---- all-gather projected hidden ----
            nc.gpsimd.collective_compute(
                "AllGather", mybir.AluOpType.bypass,
                replica_groups=[list(range(N_CORES))],
                ins=[hg_in.opt()], outs=[hg_out.opt()],
            )

            # PE warm-up: slow f32 (4-pass) junk matmuls keep the HAM clock
            # gate at 2.4 GHz through the barrier+AllGather stall. Results
            # land in the pm banks; the first real matmul's start=True
            # clears them.
            for j in range(N_DUMMY):
                nc.tensor.matmul(pms[j % MT][:], ones[:, 0:128],
                                 ones[:, 0:F], start=True, stop=True)

            # A-tile loads (sync queue), prefetched ahead of the loop
            a_raws = []
            for kt in range(KT):
                a_raw = p.tile([128, NB], f32, tag="araw", bufs=8,
                               name=f"araw{kt}")
                nc.sync.dma_start(a_raw[:], at[kt * 128:(kt + 1) * 128, :])
                a_raws.append(a_raw)

            for kt in range(KT):
                # one DVE op: a_bin = (a_raw != 0) * 1.0 (bf16, exact),
                # accum_out = free-dim sums = partial column sums of A
                a_bin = p.tile([128, NB], bf16, tag="abin", bufs=36,
                               name=f"abin{kt}")
                nc.vector.scalar_tensor_tensor(
                    a_bin[:], a_raws[kt][:], 0.0, ones[:],
                    mybir.AluOpType.not_equal, mybir.AluOpType.mult,
                    accum_out=cs[:, kt:kt + 1])
                h_t = p.tile([128, F], bf16, tag="ht", bufs=12,
                             name=f"ht{kt}")
                nc.scalar.dma_start(h_t[:],
                                    hg_out[kt * 128:(kt + 1) * 128, :])
                for m in range(MT):
                    nc.tensor.matmul(
                        pms[m][:],
                        a_bin[:, m * 128:(m + 1) * 128],
                        h_t[:],
                        start=(kt == 0), stop=(kt == KT - 1))

            # ---- phase 3: degree + normalize ----
            nc.gpsimd.dma_start(dg_in[:], cs[:])
            nc.gpsimd.collective_compute(
                "AllGather", mybir.AluOpType.bypass,
                replica_groups=[list(range(N_CORES))],
                ins=[dg_in.opt()], outs=[dg_out.opt()],
            )
            # pull each rank's partial for OUR column block (kt = pid*8+m)
            deg = p.tile([128, MT], f32)
            prt0 = p.tile([128, MT], f32, tag="prt", bufs=4, name="prt0")
            nc.gpsimd.dma_start(prt0[:], dg_out[0:128, bass.ts(pid, MT)])
            nc.vector.tensor_copy(deg[:], prt0[:])
            for r in range(1, N_CORES):
                prt = p.tile([128, MT], f32, tag="prt", bufs=4,
                             name=f"prt{r}")
                nc.gpsimd.dma_start(
                    prt[:], dg_out[r * 128:(r + 1) * 128, bass.ts(pid, MT)])
                nc.vector.tensor_tensor(deg[:], deg[:], prt[:],
                                        mybir.AluOpType.add)
            rdeg = p.tile([128, MT], f32)
            nc.vector.reciprocal(rdeg[:], deg[:])

            for m in range(MT):
                o_sb = p.tile([128, F], f32, tag="osb", bufs=2,
                              name=f"osb{m}")
                nc.vector.tensor_scalar(o_sb[:], pms[m][:],
                                        rdeg[:, m:m + 1], None,
                                        mybir.AluOpType.mult)
                nc.sync.dma_start(out[m * 128:(m + 1) * 128, :], o_sb[:])

    nc.compile()
    return nc


def _get_nc():
    if "nc" not in _cached:
        _cached["nc"] = _build()
    return _cached["nc"]


def kernel(input_features, adj, W, b):
    from concourse.bass_utils import run_bass_kernel_spmd

    x = np.ascontiguousarray(np.asarray(input_features, dtype=np.float32))
    a = np.asarray(adj, dtype=np.float32)
    wm = np.ascontiguousarray(np.asarray(W, dtype=np.float32))
    bv = np.asarray(b, dtype=np.float32)
    bfull = np.ascontiguousarray(np.broadcast_to(bv, (128, F)))

    nc = _get_nc()
    in_maps = []
    for k in range(N_CORES):
        blk = slice(k * NB, (k + 1) * NB)
        in_maps.append({
            "at": np.ascontiguousarray(a[blk, :].T),
            "xt": np.ascontiguousarray(x[blk, :].T),
            "w": wm,
            "bfull": bfull,
        })
    res = run_bass_kernel_spmd(nc, in_maps, core_ids=list(range(N_CORES)))
    return np.concatenate([res.results[k]["out"] for k in range(N_CORES)],
                          axis=0)
